# revision 2
# baseline (speedup 1.0000x reference)
"""Causal self-attention Trainium2 kernel (8 NeuronCores, tensor-parallel) v2.

Sharding: core c handles batch b=c//2 and heads [8*(c%2) .. 8*(c%2)+8).
Each core computes QKV for its 8 heads, causal flash-style attention in
transposed (S^T) layout, and a partial output projection over its 512
head-features. Host sums the two half partials per batch and adds b_proj.

v2 changes vs baseline:
- QKV is computed per 512-token block and interleaved into the attention
  group loop as PE filler work (attention is ACT/exp-bound per group).
- V tiles carry a 64-wide ones block per head so the PV matmul's unused
  stationary columns produce the softmax denominator replicated on
  partitions 64..127 (no gpsimd broadcast, no [1,N] reciprocal).
- reciprocal_approx_fast (1 DVE instr, ~1 cyc/elem) replaces the exact
  iterative-divide reciprocal (~6 cyc/elem on HW).
- kq bias-adds run on the Scalar engine (idle during QKV) as
  Identity-with-bias activations instead of DVE tensor_scalar ops.
- Weight DMAs hoisted out of the rep loop (loaded once, reused).

Datapath is bf16 (fp32 PSUM accumulation).

Self-contained: only imports concourse / jax / numpy / ml_dtypes.
"""
import numpy as np
import ml_dtypes
import jax
from jax.sharding import Mesh, PartitionSpec
from jax.experimental.shard_map import shard_map

import concourse.bacc as bacc
import concourse.mybir as mybir
import concourse.tile as tile
from concourse.bass2jax import (_bass_exec_p, install_neuronx_cc_hook,
                                partition_id_tensor)

B, SEQ, D = 4, 2048, 1024
H, HD = 16, 64
NCORES = 8
P = 128
QB = 512            # q block
NQB = SEQ // QB     # 4
NKT = SEQ // P      # 16 k tiles
PAIRS = 4           # head pairs per core
FP32 = mybir.dt.float32
BF16 = mybir.dt.bfloat16
AF = mybir.ActivationFunctionType
ALU = mybir.AluOpType
BF = ml_dtypes.bfloat16


def build_nc(nreps: int = 1, debug: bool = False):
    nc = bacc.Bacc()
    xT_e = nc.dram_tensor("xT", [D, SEQ], BF16, kind="ExternalInput")
    wkq_e = nc.dram_tensor("wkq", [PAIRS, 8, P, 256], BF16, kind="ExternalInput")
    bkq_e = nc.dram_tensor("bkq", [P, 8], FP32, kind="ExternalInput")
    wv_e = nc.dram_tensor("wv", [8, P, 512], BF16, kind="ExternalInput")
    bvo_e = nc.dram_tensor("bvo", [1, 1024], FP32, kind="ExternalInput")
    wproj_e = nc.dram_tensor("wproj", [PAIRS, P, D], BF16, kind="ExternalInput")
    mask_e = nc.dram_tensor("mask", [P, P], BF16, kind="ExternalInput")
    y_e = nc.dram_tensor("y", [SEQ, D], FP32, kind="ExternalOutput")
    if debug:
        sad = nc.dram_tensor("sad", [PAIRS, NQB, P, QB], FP32,
                             kind="ExternalOutput")

    with tile.TileContext(nc) as tc:
        with (
            tc.tile_pool(name="const", bufs=1) as constp,
            tc.tile_pool(name="wts", bufs=1) as wts,
            tc.tile_pool(name="kqv", bufs=1) as kqv,
            tc.tile_pool(name="xt", bufs=2) as xtp,
            tc.tile_pool(name="sa", bufs=2) as sap,
            tc.tile_pool(name="ep", bufs=6) as ep,
            tc.tile_pool(name="rc", bufs=4) as rcp,
            tc.tile_pool(name="yp", bufs=2) as yp,
            tc.tile_pool(name="ps", bufs=2, space="PSUM") as psp,   # scores 2x2 banks
            tc.tile_pool(name="pv", bufs=2, space="PSUM") as pvp,   # pv accum 2 banks
            tc.tile_pool(name="pj", bufs=2, space="PSUM") as pjp,   # kq/v/proj 2 banks
        ):
            # ---------- constants ----------
            mask_t = constp.tile([P, P], BF16, name="mask_t")
            nc.sync.dma_start(out=mask_t[:], in_=mask_e[:])
            bkq_t = constp.tile([P, 8], FP32, name="bkq_t")
            nc.sync.dma_start(out=bkq_t[:], in_=bkq_e[:])
            bvo_t = constp.tile([1, 1024], FP32, name="bvo_t")
            nc.sync.dma_start(out=bvo_t[:], in_=bvo_e[:])
            bv_bc = constp.tile([P, 1024], FP32, name="bv_bc")
            nc.gpsimd.partition_broadcast(bv_bc[:], bvo_t[:], channels=P)

            # ---------- weights (loaded once, reused across reps) ----------
            # DMA order is priority order: pair-0 kq weights, then x block 0
            # (issued by the first qkv chunk below), then the rest.
            wkq_t = [[wts.tile([P, 256], BF16, name=f"wkq_{p}_{c}")
                      for c in range(8)] for p in range(PAIRS)]
            wv_t = [wts.tile([P, 512], BF16, name=f"wv_{c}") for c in range(8)]
            wproj_t = [wts.tile([P, D], BF16, name=f"wproj_{p}")
                       for p in range(PAIRS)]
            for c in range(8):
                nc.sync.dma_start(out=wkq_t[0][c][:], in_=wkq_e[0, c])

            def dma_rest_of_weights():
                for p in range(1, PAIRS):
                    for c in range(8):
                        nc.sync.dma_start(out=wkq_t[p][c][:], in_=wkq_e[p, c])
                for c in range(8):
                    nc.sync.dma_start(out=wv_t[c][:], in_=wv_e[c])
                for p in range(PAIRS):
                    nc.sync.dma_start(out=wproj_t[p][:], in_=wproj_e[p])

            # ---------- persistent kqv tiles ----------
            # kT/qT: per (pair, block) [128, 512]; partitions 0:64 head A dims,
            # 64:128 head B dims.
            kT = [[kqv.tile([P, QB], BF16, name=f"kT{p}_{nb}")
                   for nb in range(NQB)] for p in range(PAIRS)]
            qT = [[kqv.tile([P, QB], BF16, name=f"qT{p}_{nb}")
                   for nb in range(NQB)] for p in range(PAIRS)]
            # v tiles: [128 kpos, 1024]; head h at cols 128h: [ones(64)|v_h(64)]
            # (ones first so the softmax denominator lands on PSUM partitions
            # 0:64 -- reciprocal_approx_fast requires base partition 0)
            vt = [kqv.tile([P, 1024], BF16, name=f"v{i}") for i in range(NKT)]
            for i in range(NKT):
                ov = vt[i][:].rearrange("p (h t d) -> p h t d", t=2, d=64)[:, :, 0, :]
                iv = bv_bc[:].rearrange("p (h t d) -> p h t d", t=2, d=64)[:, :, 0, :]
                nc.vector.tensor_copy(ov, iv)

            # ---------- qkv chunk builders ----------
            def make_qkv_chunks(r, nb):
                """Return list of closures, each emitting one PE-sized chunk
                of the QKV computation for token block nb."""
                xts = [None] * 8

                def dma_x():
                    for c in range(8):
                        t = xtp.tile([P, QB], BF16, tag=f"x{c}",
                                     name=f"x{r}_{nb}_{c}")
                        nc.sync.dma_start(
                            out=t[:],
                            in_=xT_e[c * P:(c + 1) * P,
                                     nb * QB:(nb + 1) * QB])
                        xts[c] = t

                def kq_chunk(p, ec):
                    def go():
                        ps = pjp.tile([P, QB], FP32, tag="pj",
                                      name=f"kq{r}_{nb}_{p}_{ec}")
                        for c in range(8):
                            nc.tensor.matmul(
                                ps[:], wkq_t[p][c][:, ec * P:(ec + 1) * P],
                                xts[c][:], start=(c == 0), stop=(c == 7))
                        dst = (kT if ec == 0 else qT)[p][nb]
                        nc.scalar.activation(
                            dst[:], ps[:], AF.Identity,
                            bias=bkq_t[:, 2 * p + ec:2 * p + ec + 1])
                    return go

                def v_chunk(ntl, half):
                    def go():
                        nt = nb * 4 + ntl
                        psv = pjp.tile([P, QB], FP32, tag="pj",
                                       name=f"v{r}_{nt}_{half}")
                        for c in range(8):
                            nc.tensor.matmul(
                                psv[:, 0:256],
                                xts[c][:, ntl * P:(ntl + 1) * P],
                                wv_t[c][:, half * 256:(half + 1) * 256],
                                start=(c == 0), stop=(c == 7))
                        ov = vt[nt][:].rearrange(
                            "p (h t d) -> p h t d", t=2, d=64
                        )[:, 4 * half:4 * half + 4, 1, :]
                        iv = bv_bc[:].rearrange(
                            "p (h t d) -> p h t d", t=2, d=64
                        )[:, 4 * half:4 * half + 4, 1, :]
                        nc.vector.tensor_tensor(
                            ov,
                            psv[:, 0:256].rearrange("p (h d) -> p h d", d=64),
                            iv, ALU.add)
                    return go

                # dep: min pair index of attn(3) that must finish before
                # this chunk may run when filling across a rep boundary
                # (kT[p][0] is read by attn(3) pair p; vt[0..3] by all pairs).
                chunks = [(-1, dma_x)]
                for p in range(PAIRS):
                    chunks.append((p, kq_chunk(p, 0)))
                    chunks.append((-1, kq_chunk(p, 1)))
                for ntl in range(4):
                    for half in range(2):
                        chunks.append((PAIRS - 1, v_chunk(ntl, half)))
                return chunks

            # ---------- attention for one q-block ----------
            def attn_block(r, j, work_q, per_slot, guard=False):
                cur_pair = [0]

                def fill(n):
                    done = 0
                    for idx in range(len(work_q)):
                        if done == n:
                            break
                        dep, fn = work_q[idx]
                        if fn is None or (guard and dep >= cur_pair[0]):
                            continue
                        fn()
                        work_q[idx] = (dep, None)
                        done += 1

                saT = []
                for p in range(PAIRS):
                    cur_pair[0] = p
                    hA, hB = 2 * p, 2 * p + 1
                    sa_t = sap.tile([P, QB], BF16, tag=f"sa{p}",
                                    name=f"sa{r}_{p}_{j}")
                    saT.append(sa_t)
                    pvA = pvp.tile([P, QB], FP32, tag="pv",
                                   name=f"pvA{r}_{p}_{j}")
                    pvB = pvp.tile([P, QB], FP32, tag="pv",
                                   name=f"pvB{r}_{p}_{j}")
                    for g in range(2 * j + 2):
                        sA = psp.tile([P, 1024], FP32, tag="s",
                                      name=f"sA{r}_{p}_{j}_{g}")
                        sB = psp.tile([P, 1024], FP32, tag="s",
                                      name=f"sB{r}_{p}_{j}_{g}")
                        for t in range(2):
                            i = 2 * g + t
                            off = P * (i - 4 * j) if i >= 4 * j else 0
                            ib, ic = i // 4, (i % 4) * P
                            osl = slice(t * QB + off, (t + 1) * QB)
                            nc.tensor.matmul(
                                sA[:, osl], kT[p][ib][0:64, ic:ic + P],
                                qT[p][j][0:64, off:QB], start=True,
                                stop=True, tile_position=(0, 0))
                            nc.tensor.matmul(
                                sB[:, osl], kT[p][ib][64:P, ic:ic + P],
                                qT[p][j][64:P, off:QB], start=True,
                                stop=True, tile_position=(64, 0))
                        eA = ep.tile([P, 1024], BF16, tag="e",
                                     name=f"eA{r}_{p}_{j}_{g}")
                        eB = ep.tile([P, 1024], BF16, tag="e",
                                     name=f"eB{r}_{p}_{j}_{g}")
                        e0 = 256 if g == 2 * j + 1 else 0
                        nc.scalar.activation(eA[:, e0:1024], sA[:, e0:1024],
                                             AF.Exp, scale=0.125)
                        nc.scalar.activation(eB[:, e0:1024], sB[:, e0:1024],
                                             AF.Exp, scale=0.125)
                        for t in range(2):
                            i = 2 * g + t
                            if i >= 4 * j:
                                c0 = t * QB + P * (i - 4 * j)
                                for e_t in (eA, eB):
                                    nc.vector.tensor_mul(
                                        e_t[:, c0:c0 + P],
                                        e_t[:, c0:c0 + P], mask_t[:])
                        for t in range(2):
                            i = 2 * g + t
                            off = P * (i - 4 * j) if i >= 4 * j else 0
                            esl = slice(t * QB + off, (t + 1) * QB)
                            st, sp = (i == 0), (i == 4 * j + 3)
                            nc.tensor.matmul(
                                pvA[:, off:QB],
                                vt[i][:, 128 * hA:128 * hA + 128],
                                eA[:, esl], start=st, stop=sp)
                            nc.tensor.matmul(
                                pvB[:, off:QB],
                                vt[i][:, 128 * hB:128 * hB + 128],
                                eB[:, esl], start=st, stop=sp)
                        fill(per_slot)
                    # ---- normalize: den is replicated on partitions 64:128
                    rcA = rcp.tile([64, QB], FP32, tag="rc",
                                   name=f"rA{r}_{p}_{j}")
                    rcB = rcp.tile([64, QB], FP32, tag="rc",
                                   name=f"rB{r}_{p}_{j}")
                    nc.vector.reciprocal_approx_fast(rcA[:], pvA[0:64, :])
                    nc.vector.reciprocal_approx_fast(rcB[:], pvB[0:64, :])
                    nc.vector.tensor_tensor(
                        sa_t[0:64, :], pvA[64:P, :], rcA[:], ALU.mult)
                    nc.vector.tensor_tensor(
                        sa_t[64:P, :], pvB[64:P, :], rcB[:], ALU.mult)
                    if debug:
                        nc.sync.dma_start(
                            out=sad[p, j],
                            in_=sa_t[:].bitcast(mybir.dt.uint16))
                # flush remaining qkv chunks before proj
                cur_pair[0] = PAIRS
                fill(len(work_q))
                # ---- projection for this q-block ----
                for half in range(2):
                    for ntl in range(4):
                        psy = pjp.tile([P, QB], FP32, tag="pj",
                                       name=f"y{r}_{j}_{half}_{ntl}")
                        for p in range(PAIRS):
                            nc.tensor.matmul(
                                psy[:],
                                saT[p][:, ntl * P:(ntl + 1) * P],
                                wproj_t[p][:, half * QB:(half + 1) * QB],
                                start=(p == 0), stop=(p == 3))
                        yt = yp.tile([P, QB], FP32, tag="y",
                                     name=f"yt{r}_{j}_{half}_{ntl}")
                        nc.vector.tensor_copy(yt[:], psy[:])
                        r0 = j * QB + ntl * P
                        nc.sync.dma_start(
                            out=y_e[r0:r0 + P,
                                    half * QB:(half + 1) * QB],
                            in_=yt[:])

            # ---------- program ----------
            # startup: qkv(0) of rep 0 stands alone
            chunks0 = make_qkv_chunks(0, 0)
            chunks0[0][1]()       # x block-0 DMA, ahead of remaining weights
            dma_rest_of_weights()
            for _, chunk in chunks0[1:]:
                chunk()
            for r in range(nreps):
                for j in range(NQB):
                    work_q, guard = [], False
                    if j + 1 < NQB:
                        work_q = make_qkv_chunks(r, j + 1)
                    elif r + 1 < nreps:
                        # Tile's dataflow scheduler serializes the next rep's
                        # kT/vt writes behind this rep's reads (WAR semaphores),
                        # so the cross-rep filler needs no ordering guard.
                        work_q = make_qkv_chunks(r + 1, 0)
                    # groups available: 4 pairs * (2j+2); chunks: 17
                    slots = 4 * (2 * j + 2)
                    per_slot = max(1, -(-len(work_q) // slots))
                    attn_block(r, j, work_q, per_slot, guard=guard)
    nc.finalize()
    return nc


def prep_core_inputs(core, x, W_kqv, b_kqv, W_proj):
    b, half = core // 2, core % 2
    heads = [8 * half + m for m in range(8)]
    xT = np.ascontiguousarray(np.asarray(x[b], np.float32).T).astype(BF)
    wkq = np.empty((PAIRS, 8, P, 256), np.float32)
    bkq = np.empty((P, 8), np.float32)
    for p in range(PAIRS):
        gA, gB = heads[2 * p], heads[2 * p + 1]
        blk = np.empty((D, 256), np.float32)
        blk[:, 0:64] = W_kqv[gA][:, 0:64]      # k_A
        blk[:, 64:128] = W_kqv[gB][:, 0:64]    # k_B
        blk[:, 128:192] = W_kqv[gA][:, 64:128]  # q_A
        blk[:, 192:256] = W_kqv[gB][:, 64:128]  # q_B
        wkq[p] = blk.reshape(8, P, 256)
        bkq[0:64, 2 * p] = b_kqv[gA][0:64]
        bkq[64:128, 2 * p] = b_kqv[gB][0:64]
        bkq[0:64, 2 * p + 1] = b_kqv[gA][64:128]
        bkq[64:128, 2 * p + 1] = b_kqv[gB][64:128]
    wv = np.zeros((D, 512), np.float32)
    bvo = np.zeros((1, 1024), np.float32)
    for m, g in enumerate(heads):
        wv[:, 64 * m:64 * m + 64] = W_kqv[g][:, 128:192]
        bvo[0, 128 * m:128 * m + 64] = 1.0
        bvo[0, 128 * m + 64:128 * m + 128] = b_kqv[g][128:192]
    wproj = np.ascontiguousarray(
        W_proj[512 * half:512 * half + 512, :]).reshape(PAIRS, P, D)
    mask = np.triu(np.ones((P, P), np.float32))  # mask[r,c]=1 iff c>=r
    return {
        "xT": xT, "wkq": wkq.astype(BF), "bkq": bkq,
        "wv": wv.reshape(8, P, 512).astype(BF), "bvo": bvo,
        "wproj": wproj.astype(BF), "mask": mask.astype(BF),
    }


class SpmdRunner:
    def __init__(self, nc, n_cores=NCORES):
        install_neuronx_cc_hook()
        self.nc = nc
        self.n_cores = n_cores
        pname = nc.partition_id_tensor.name if nc.partition_id_tensor else None
        in_names, out_names, out_avals, zero_outs = [], [], [], []
        for alloc in nc.m.functions[0].allocations:
            if not isinstance(alloc, mybir.MemoryLocationSet):
                continue
            name = alloc.memorylocations[0].name
            if alloc.kind == "ExternalInput":
                if name != pname:
                    in_names.append(name)
            elif alloc.kind == "ExternalOutput":
                out_names.append(name)
                shape = tuple(alloc.tensor_shape)
                dtype = mybir.dt.np(alloc.dtype)
                out_avals.append(jax.core.ShapedArray(shape, dtype))
                zero_outs.append(np.zeros(shape, dtype))
        self.in_names, self.out_names = in_names, out_names
        self.out_avals, self.zero_outs = out_avals, zero_outs
        n_params = len(in_names)
        all_in = in_names + out_names + ([pname] if pname else [])

        def _body(*args):
            operands = list(args)
            if pname is not None:
                operands.append(partition_id_tensor())
            outs = _bass_exec_p.bind(
                *operands, out_avals=tuple(out_avals),
                in_names=tuple(all_in), out_names=tuple(out_names),
                lowering_input_output_aliases=(),
                sim_require_finite=False, sim_require_nnan=False, nc=nc)
            return tuple(outs)

        devices = jax.devices()[:n_cores]
        self.mesh = Mesh(np.asarray(devices), ("core",))
        nin = n_params + len(out_names)
        self.sharded = jax.jit(
            shard_map(_body, mesh=self.mesh,
                      in_specs=(PartitionSpec("core"),) * nin,
                      out_specs=(PartitionSpec("core"),) * len(out_names),
                      check_rep=False),
            keep_unused=True)
        self._dev_args = None

    def put(self, in_maps):
        n = self.n_cores
        arrs = [np.concatenate([np.asarray(in_maps[c][nm]) for c in range(n)],
                               axis=0) for nm in self.in_names]
        arrs += [np.zeros((n * z.shape[0], *z.shape[1:]), z.dtype)
                 for z in self.zero_outs]
        sh = jax.sharding.NamedSharding(self.mesh, PartitionSpec("core"))
        self._dev_args = [jax.device_put(a, sh) for a in arrs]

    def run(self):
        out_arrs = self.sharded(*self._dev_args)
        jax.block_until_ready(out_arrs)
        n = self.n_cores
        return [
            {nm: np.asarray(out_arrs[i]).reshape(n, *self.out_avals[i].shape)[c]
             for i, nm in enumerate(self.out_names)}
            for c in range(n)
        ]


_CACHE = {}


def kernel(x, W_kqv, b_kqv, W_proj, b_proj):
    x = np.asarray(x, np.float32)
    W_kqv = np.asarray(W_kqv, np.float32)
    b_kqv = np.asarray(b_kqv, np.float32)
    W_proj = np.asarray(W_proj, np.float32)
    b_proj = np.asarray(b_proj, np.float32)
    if "r" not in _CACHE:
        _CACHE["r"] = SpmdRunner(build_nc(1))
    r = _CACHE["r"]
    in_maps = [prep_core_inputs(c, x, W_kqv, b_kqv, W_proj)
               for c in range(NCORES)]
    r.put(in_maps)
    res = r.run()
    y = np.empty((B, SEQ, D), np.float32)
    for b in range(B):
        y[b] = res[2 * b]["y"] + res[2 * b + 1]["y"] + b_proj[None, :]
    return y


# revision 3
# speedup vs baseline: 1.2688x; 1.2688x over previous
"""Causal self-attention Trainium2 kernel (8 NeuronCores, tensor-parallel) v2.

Sharding: core c handles batch b=c//2 and heads [8*(c%2) .. 8*(c%2)+8).
Each core computes QKV for its 8 heads, causal flash-style attention in
transposed (S^T) layout, and a partial output projection over its 512
head-features. Host sums the two half partials per batch and adds b_proj.

v2 changes vs baseline:
- QKV is computed per 512-token block and interleaved into the attention
  group loop as PE filler work (attention is ACT/exp-bound per group).
- V tiles carry a 64-wide ones block per head so the PV matmul's unused
  stationary columns produce the softmax denominator replicated on
  partitions 64..127 (no gpsimd broadcast, no [1,N] reciprocal).
- reciprocal_approx_fast (1 DVE instr, ~1 cyc/elem) replaces the exact
  iterative-divide reciprocal (~6 cyc/elem on HW).
- kq bias-adds run on the Scalar engine (idle during QKV) as
  Identity-with-bias activations instead of DVE tensor_scalar ops.
- Weight DMAs hoisted out of the rep loop (loaded once, reused).

Datapath is bf16 (fp32 PSUM accumulation).

Self-contained: only imports concourse / jax / numpy / ml_dtypes.
"""
import numpy as np
import ml_dtypes
import jax
from jax.sharding import Mesh, PartitionSpec
from jax.experimental.shard_map import shard_map

import concourse.bacc as bacc
import concourse.mybir as mybir
import concourse.tile as tile
from concourse.bass2jax import (_bass_exec_p, install_neuronx_cc_hook,
                                partition_id_tensor)

B, SEQ, D = 4, 2048, 1024
H, HD = 16, 64
NCORES = 8
P = 128
QB = 512            # q block
NQB = SEQ // QB     # 4
NKT = SEQ // P      # 16 k tiles
PAIRS = 4           # head pairs per core
FP32 = mybir.dt.float32
BF16 = mybir.dt.bfloat16
AF = mybir.ActivationFunctionType
ALU = mybir.AluOpType
BF = ml_dtypes.bfloat16


def build_nc(nreps: int = 1, debug: bool = False):
    nc = bacc.Bacc()
    xT_e = nc.dram_tensor("xT", [D, SEQ], BF16, kind="ExternalInput")
    wkq_e = nc.dram_tensor("wkq", [PAIRS, 8, P, 256], BF16, kind="ExternalInput")
    bkq_e = nc.dram_tensor("bkq", [P, 8], FP32, kind="ExternalInput")
    wv_e = nc.dram_tensor("wv", [8, P, 512], BF16, kind="ExternalInput")
    bvo_e = nc.dram_tensor("bvo", [1, 1024], FP32, kind="ExternalInput")
    wproj_e = nc.dram_tensor("wproj", [PAIRS, P, D], BF16, kind="ExternalInput")
    mask_e = nc.dram_tensor("mask", [P, P], BF16, kind="ExternalInput")
    y_e = nc.dram_tensor("y", [SEQ, D], FP32, kind="ExternalOutput")
    if debug:
        sad = nc.dram_tensor("sad", [PAIRS, NQB, P, QB], FP32,
                             kind="ExternalOutput")

    with tile.TileContext(nc) as tc:
        with (
            tc.tile_pool(name="const", bufs=1) as constp,
            tc.tile_pool(name="wts", bufs=1) as wts,
            tc.tile_pool(name="kqv", bufs=1) as kqv,
            tc.tile_pool(name="xt", bufs=2) as xtp,
            tc.tile_pool(name="sa", bufs=2) as sap,
            tc.tile_pool(name="ep", bufs=6) as ep,
            tc.tile_pool(name="rc", bufs=4) as rcp,
            tc.tile_pool(name="yp", bufs=2) as yp,
            tc.tile_pool(name="ps", bufs=2, space="PSUM") as psp,   # scores 2x2 banks
            tc.tile_pool(name="pv", bufs=2, space="PSUM") as pvp,   # pv accum 2 banks
            tc.tile_pool(name="pj", bufs=2, space="PSUM") as pjp,   # kq/v/proj 2 banks
        ):
            # ---------- constants ----------
            mask_t = constp.tile([P, P], BF16, name="mask_t")
            nc.sync.dma_start(out=mask_t[:], in_=mask_e[:])
            bkq_t = constp.tile([P, 8], FP32, name="bkq_t")
            nc.sync.dma_start(out=bkq_t[:], in_=bkq_e[:])
            bvo_t = constp.tile([1, 1024], FP32, name="bvo_t")
            nc.sync.dma_start(out=bvo_t[:], in_=bvo_e[:])
            bv_bc = constp.tile([P, 1024], FP32, name="bv_bc")
            nc.gpsimd.partition_broadcast(bv_bc[:], bvo_t[:], channels=P)

            # ---------- weights (loaded once, reused across reps) ----------
            # DMA order is priority order: pair-0 kq weights, then x block 0
            # (issued by the first qkv chunk below), then the rest.
            wkq_t = [[wts.tile([P, 256], BF16, name=f"wkq_{p}_{c}")
                      for c in range(8)] for p in range(PAIRS)]
            wv_t = [wts.tile([P, 512], BF16, name=f"wv_{c}") for c in range(8)]
            wproj_t = [wts.tile([P, D], BF16, name=f"wproj_{p}")
                       for p in range(PAIRS)]
            for c in range(8):
                nc.sync.dma_start(out=wkq_t[0][c][:], in_=wkq_e[0, c])

            def dma_rest_of_weights():
                for p in range(1, PAIRS):
                    for c in range(8):
                        nc.sync.dma_start(out=wkq_t[p][c][:], in_=wkq_e[p, c])
                for c in range(8):
                    nc.sync.dma_start(out=wv_t[c][:], in_=wv_e[c])
                for p in range(PAIRS):
                    nc.sync.dma_start(out=wproj_t[p][:], in_=wproj_e[p])

            # ---------- persistent kqv tiles ----------
            # kT/qT: per (pair, block) [128, 512]; partitions 0:64 head A dims,
            # 64:128 head B dims.
            kT = [[kqv.tile([P, QB], BF16, name=f"kT{p}_{nb}")
                   for nb in range(NQB)] for p in range(PAIRS)]
            qT = [[kqv.tile([P, QB], BF16, name=f"qT{p}_{nb}")
                   for nb in range(NQB)] for p in range(PAIRS)]
            # v tiles: [128 kpos, 1024]; head h at cols 128h: [ones(64)|v_h(64)]
            # (ones first so the softmax denominator lands on PSUM partitions
            # 0:64 -- reciprocal_approx_fast requires base partition 0)
            vt = [kqv.tile([P, 1024], BF16, name=f"v{i}") for i in range(NKT)]
            for i in range(NKT):
                ov = vt[i][:].rearrange("p (h t d) -> p h t d", t=2, d=64)[:, :, 0, :]
                iv = bv_bc[:].rearrange("p (h t d) -> p h t d", t=2, d=64)[:, :, 0, :]
                nc.vector.tensor_copy(ov, iv)

            # ---------- qkv chunk builders ----------
            def make_qkv_chunks(r, nb):
                """Return list of closures, each emitting one PE-sized chunk
                of the QKV computation for token block nb."""
                xts = [None] * 8

                def dma_x():
                    for c in range(8):
                        t = xtp.tile([P, QB], BF16, tag=f"x{c}",
                                     name=f"x{r}_{nb}_{c}")
                        nc.sync.dma_start(
                            out=t[:],
                            in_=xT_e[c * P:(c + 1) * P,
                                     nb * QB:(nb + 1) * QB])
                        xts[c] = t

                def kq_chunk(p, ec):
                    def go():
                        ps = pjp.tile([P, QB], FP32, tag="pj",
                                      name=f"kq{r}_{nb}_{p}_{ec}")
                        for c in range(8):
                            nc.tensor.matmul(
                                ps[:], wkq_t[p][c][:, ec * P:(ec + 1) * P],
                                xts[c][:], start=(c == 0), stop=(c == 7))
                        dst = (kT if ec == 0 else qT)[p][nb]
                        nc.scalar.activation(
                            dst[:], ps[:], AF.Identity,
                            bias=bkq_t[:, 2 * p + ec:2 * p + ec + 1])
                    return go

                def v_chunk(ntl, half):
                    def go():
                        nt = nb * 4 + ntl
                        psv = pjp.tile([P, QB], FP32, tag="pj",
                                       name=f"v{r}_{nt}_{half}")
                        for c in range(8):
                            nc.tensor.matmul(
                                psv[:, 0:256],
                                xts[c][:, ntl * P:(ntl + 1) * P],
                                wv_t[c][:, half * 256:(half + 1) * 256],
                                start=(c == 0), stop=(c == 7))
                        ov = vt[nt][:].rearrange(
                            "p (h t d) -> p h t d", t=2, d=64
                        )[:, 4 * half:4 * half + 4, 1, :]
                        iv = bv_bc[:].rearrange(
                            "p (h t d) -> p h t d", t=2, d=64
                        )[:, 4 * half:4 * half + 4, 1, :]
                        nc.vector.tensor_tensor(
                            ov,
                            psv[:, 0:256].rearrange("p (h d) -> p h d", d=64),
                            iv, ALU.add)
                    return go

                # dep: min pair index of attn(3) that must finish before
                # this chunk may run when filling across a rep boundary
                # (kT[p][0] is read by attn(3) pair p; vt[0..3] by all pairs).
                chunks = [(-1, dma_x)]
                for p in range(PAIRS):
                    chunks.append((p, kq_chunk(p, 0)))
                    chunks.append((-1, kq_chunk(p, 1)))
                for ntl in range(4):
                    for half in range(2):
                        chunks.append((PAIRS - 1, v_chunk(ntl, half)))
                return chunks

            # ---------- attention for one q-block ----------
            def attn_block(r, j, work_q, per_slot, guard=False):
                cur_pair = [0]

                def fill(n):
                    done = 0
                    for idx in range(len(work_q)):
                        if done == n:
                            break
                        dep, fn = work_q[idx]
                        if fn is None or (guard and dep >= cur_pair[0]):
                            continue
                        fn()
                        work_q[idx] = (dep, None)
                        done += 1

                saT = []
                for p in range(PAIRS):
                    cur_pair[0] = p
                    hA, hB = 2 * p, 2 * p + 1
                    sa_t = sap.tile([P, QB], BF16, tag=f"sa{p}",
                                    name=f"sa{r}_{p}_{j}")
                    saT.append(sa_t)
                    pvA = pvp.tile([P, QB], FP32, tag="pv",
                                   name=f"pvA{r}_{p}_{j}")
                    pvB = pvp.tile([P, QB], FP32, tag="pv",
                                   name=f"pvB{r}_{p}_{j}")
                    for g in range(2 * j + 2):
                        sA = psp.tile([P, 1024], FP32, tag="s",
                                      name=f"sA{r}_{p}_{j}_{g}")
                        sB = psp.tile([P, 1024], FP32, tag="s",
                                      name=f"sB{r}_{p}_{j}_{g}")
                        for t in range(2):
                            i = 2 * g + t
                            off = P * (i - 4 * j) if i >= 4 * j else 0
                            ib, ic = i // 4, (i % 4) * P
                            osl = slice(t * QB + off, (t + 1) * QB)
                            nc.tensor.matmul(
                                sA[:, osl], kT[p][ib][0:64, ic:ic + P],
                                qT[p][j][0:64, off:QB], start=True,
                                stop=True, tile_position=(0, 0))
                            nc.tensor.matmul(
                                sB[:, osl], kT[p][ib][64:P, ic:ic + P],
                                qT[p][j][64:P, off:QB], start=True,
                                stop=True, tile_position=(64, 0))
                        eA = ep.tile([P, 1024], BF16, tag="e",
                                     name=f"eA{r}_{p}_{j}_{g}")
                        eB = ep.tile([P, 1024], BF16, tag="e",
                                     name=f"eB{r}_{p}_{j}_{g}")
                        e0 = 256 if g == 2 * j + 1 else 0
                        nc.scalar.activation(eA[:, e0:1024], sA[:, e0:1024],
                                             AF.Exp, scale=0.125)
                        nc.scalar.activation(eB[:, e0:1024], sB[:, e0:1024],
                                             AF.Exp, scale=0.125)
                        for t in range(2):
                            i = 2 * g + t
                            if i >= 4 * j:
                                c0 = t * QB + P * (i - 4 * j)
                                for e_t in (eA, eB):
                                    nc.vector.tensor_mul(
                                        e_t[:, c0:c0 + P],
                                        e_t[:, c0:c0 + P], mask_t[:])
                        for t in range(2):
                            i = 2 * g + t
                            off = P * (i - 4 * j) if i >= 4 * j else 0
                            esl = slice(t * QB + off, (t + 1) * QB)
                            st, sp = (i == 0), (i == 4 * j + 3)
                            nc.tensor.matmul(
                                pvA[:, off:QB],
                                vt[i][:, 128 * hA:128 * hA + 128],
                                eA[:, esl], start=st, stop=sp)
                            nc.tensor.matmul(
                                pvB[:, off:QB],
                                vt[i][:, 128 * hB:128 * hB + 128],
                                eB[:, esl], start=st, stop=sp)
                        fill(per_slot)
                    # ---- normalize: den is replicated on partitions 64:128
                    rcA = rcp.tile([64, QB], FP32, tag="rc",
                                   name=f"rA{r}_{p}_{j}")
                    rcB = rcp.tile([64, QB], FP32, tag="rc",
                                   name=f"rB{r}_{p}_{j}")
                    nc.vector.reciprocal_approx_fast(rcA[:], pvA[0:64, :])
                    nc.vector.reciprocal_approx_fast(rcB[:], pvB[0:64, :])
                    nc.vector.tensor_tensor(
                        sa_t[0:64, :], pvA[64:P, :], rcA[:], ALU.mult)
                    nc.vector.tensor_tensor(
                        sa_t[64:P, :], pvB[64:P, :], rcB[:], ALU.mult)
                    if debug:
                        nc.sync.dma_start(
                            out=sad[p, j],
                            in_=sa_t[:].bitcast(mybir.dt.uint16))
                # flush remaining qkv chunks before proj
                cur_pair[0] = PAIRS
                fill(len(work_q))
                # ---- projection for this q-block ----
                for half in range(2):
                    for ntl in range(4):
                        psy = pjp.tile([P, QB], FP32, tag="pj",
                                       name=f"y{r}_{j}_{half}_{ntl}")
                        for p in range(PAIRS):
                            nc.tensor.matmul(
                                psy[:],
                                saT[p][:, ntl * P:(ntl + 1) * P],
                                wproj_t[p][:, half * QB:(half + 1) * QB],
                                start=(p == 0), stop=(p == 3))
                        yt = yp.tile([P, QB], FP32, tag="y",
                                     name=f"yt{r}_{j}_{half}_{ntl}")
                        nc.vector.tensor_copy(yt[:], psy[:])
                        r0 = j * QB + ntl * P
                        nc.sync.dma_start(
                            out=y_e[r0:r0 + P,
                                    half * QB:(half + 1) * QB],
                            in_=yt[:])

            # ---------- program ----------
            # startup: qkv(0) of rep 0 stands alone
            chunks0 = make_qkv_chunks(0, 0)
            chunks0[0][1]()       # x block-0 DMA, ahead of remaining weights
            dma_rest_of_weights()
            for _, chunk in chunks0[1:]:
                chunk()
            for r in range(nreps):
                for j in range(NQB):
                    work_q, guard = [], False
                    if j + 1 < NQB:
                        work_q = make_qkv_chunks(r, j + 1)
                    elif r + 1 < nreps:
                        work_q = make_qkv_chunks(r + 1, 0)
                        guard = True  # next rep overwrites tiles attn(3) reads
                    # groups available: 4 pairs * (2j+2); chunks: 17
                    slots = 4 * (2 * j + 2)
                    per_slot = max(1, -(-len(work_q) // slots))
                    attn_block(r, j, work_q, per_slot, guard=guard)
    nc.finalize()
    return nc


def prep_core_inputs(core, x, W_kqv, b_kqv, W_proj):
    b, half = core // 2, core % 2
    heads = [8 * half + m for m in range(8)]
    xT = np.ascontiguousarray(np.asarray(x[b], np.float32).T).astype(BF)
    wkq = np.empty((PAIRS, 8, P, 256), np.float32)
    bkq = np.empty((P, 8), np.float32)
    for p in range(PAIRS):
        gA, gB = heads[2 * p], heads[2 * p + 1]
        blk = np.empty((D, 256), np.float32)
        blk[:, 0:64] = W_kqv[gA][:, 0:64]      # k_A
        blk[:, 64:128] = W_kqv[gB][:, 0:64]    # k_B
        blk[:, 128:192] = W_kqv[gA][:, 64:128]  # q_A
        blk[:, 192:256] = W_kqv[gB][:, 64:128]  # q_B
        wkq[p] = blk.reshape(8, P, 256)
        bkq[0:64, 2 * p] = b_kqv[gA][0:64]
        bkq[64:128, 2 * p] = b_kqv[gB][0:64]
        bkq[0:64, 2 * p + 1] = b_kqv[gA][64:128]
        bkq[64:128, 2 * p + 1] = b_kqv[gB][64:128]
    wv = np.zeros((D, 512), np.float32)
    bvo = np.zeros((1, 1024), np.float32)
    for m, g in enumerate(heads):
        wv[:, 64 * m:64 * m + 64] = W_kqv[g][:, 128:192]
        bvo[0, 128 * m:128 * m + 64] = 1.0
        bvo[0, 128 * m + 64:128 * m + 128] = b_kqv[g][128:192]
    wproj = np.ascontiguousarray(
        W_proj[512 * half:512 * half + 512, :]).reshape(PAIRS, P, D)
    mask = np.triu(np.ones((P, P), np.float32))  # mask[r,c]=1 iff c>=r
    return {
        "xT": xT, "wkq": wkq.astype(BF), "bkq": bkq,
        "wv": wv.reshape(8, P, 512).astype(BF), "bvo": bvo,
        "wproj": wproj.astype(BF), "mask": mask.astype(BF),
    }


class SpmdRunner:
    def __init__(self, nc, n_cores=NCORES):
        install_neuronx_cc_hook()
        self.nc = nc
        self.n_cores = n_cores
        pname = nc.partition_id_tensor.name if nc.partition_id_tensor else None
        in_names, out_names, out_avals, zero_outs = [], [], [], []
        for alloc in nc.m.functions[0].allocations:
            if not isinstance(alloc, mybir.MemoryLocationSet):
                continue
            name = alloc.memorylocations[0].name
            if alloc.kind == "ExternalInput":
                if name != pname:
                    in_names.append(name)
            elif alloc.kind == "ExternalOutput":
                out_names.append(name)
                shape = tuple(alloc.tensor_shape)
                dtype = mybir.dt.np(alloc.dtype)
                out_avals.append(jax.core.ShapedArray(shape, dtype))
                zero_outs.append(np.zeros(shape, dtype))
        self.in_names, self.out_names = in_names, out_names
        self.out_avals, self.zero_outs = out_avals, zero_outs
        n_params = len(in_names)
        all_in = in_names + out_names + ([pname] if pname else [])

        def _body(*args):
            operands = list(args)
            if pname is not None:
                operands.append(partition_id_tensor())
            outs = _bass_exec_p.bind(
                *operands, out_avals=tuple(out_avals),
                in_names=tuple(all_in), out_names=tuple(out_names),
                lowering_input_output_aliases=(),
                sim_require_finite=False, sim_require_nnan=False, nc=nc)
            return tuple(outs)

        devices = jax.devices()[:n_cores]
        self.mesh = Mesh(np.asarray(devices), ("core",))
        nin = n_params + len(out_names)
        self.sharded = jax.jit(
            shard_map(_body, mesh=self.mesh,
                      in_specs=(PartitionSpec("core"),) * nin,
                      out_specs=(PartitionSpec("core"),) * len(out_names),
                      check_rep=False),
            keep_unused=True)
        self._dev_args = None

    def put(self, in_maps):
        n = self.n_cores
        arrs = [np.concatenate([np.asarray(in_maps[c][nm]) for c in range(n)],
                               axis=0) for nm in self.in_names]
        arrs += [np.zeros((n * z.shape[0], *z.shape[1:]), z.dtype)
                 for z in self.zero_outs]
        sh = jax.sharding.NamedSharding(self.mesh, PartitionSpec("core"))
        self._dev_args = [jax.device_put(a, sh) for a in arrs]

    def run(self):
        out_arrs = self.sharded(*self._dev_args)
        jax.block_until_ready(out_arrs)
        n = self.n_cores
        return [
            {nm: np.asarray(out_arrs[i]).reshape(n, *self.out_avals[i].shape)[c]
             for i, nm in enumerate(self.out_names)}
            for c in range(n)
        ]


_CACHE = {}


def kernel(x, W_kqv, b_kqv, W_proj, b_proj):
    x = np.asarray(x, np.float32)
    W_kqv = np.asarray(W_kqv, np.float32)
    b_kqv = np.asarray(b_kqv, np.float32)
    W_proj = np.asarray(W_proj, np.float32)
    b_proj = np.asarray(b_proj, np.float32)
    if "r" not in _CACHE:
        _CACHE["r"] = SpmdRunner(build_nc(1))
    r = _CACHE["r"]
    in_maps = [prep_core_inputs(c, x, W_kqv, b_kqv, W_proj)
               for c in range(NCORES)]
    r.put(in_maps)
    res = r.run()
    y = np.empty((B, SEQ, D), np.float32)
    for b in range(B):
        y[b] = res[2 * b]["y"] + res[2 * b + 1]["y"] + b_proj[None, :]
    return y
